# revision 1
# baseline (speedup 1.0000x reference)
"""DualStreamEncoderAttention Trainium2 kernel.

Sharding: 8 cores = 4 samples x 2 head-groups (8 heads each). Each core
computes, for its sample, both streams' LN+QKV(+RoPE) for its 8 heads,
cross-stream attention (KV concat is per-sample, head sharding is clean),
and a partial out-projection over its heads' rows of Wout. The host sums
the two partial projections per sample. No collectives; pure SPMD.

Layout strategy on-core (S=1024, H=1024, D=64, 8 local heads):
  - x [S,H] is PE-transposed once per stream to xT [H,S]; the LayerNorm is
    folded algebraically: LN(x)@W = r*(x@W') - (r*mu)*colsum(W') + beta@W
    with W' = gamma*W. r is folded into xT at transpose-eviction time
    (xhat = xT * rbar), the rank-2 correction is an extra K=2 matmul into
    the same PSUM accumulation.
  - Q,K are produced transposed (qkT: [n,s], n=(q|k, head, d)) which is
    exactly the [d, s]-per-head layout attention needs; V is produced
    natural [s, n] (x̂T used as the stationary operand), cast to bf16 with
    a ones column appended per head (softmax denominator trick).
  - RoPE in transposed layout: the d<->d+32 rotation is a partition shuffle
    done with SBUF->SBUF DMA copies; sin tables are pre-negated on the host
    so the shuffled copy needs no sign flip.
  - scoresT [t, s] per head: K=64 matmuls, head pairs row-packed into the
    PE array via base-partition 0/64. exp runs on the Scalar engine
    directly from PSUM ([128,1024] ops, scale=1/8 folded in), output bf16.
  - PV: stationary [t-chunk, 65] = [V_head | 1], accumulated over 16
    t-chunks into [65,512] PSUM; row 64 = softmax denominator.
  - Normalization is folded into the attention drain: per [65,512]
    accumulator, reciprocal of the denominator row (DVE), gpsimd
    partition_broadcast to 64 partitions, and a multiply-eviction that
    writes the normalized attnT pair-tile directly.
  - Out-projection accumulates pair-tiles [128=2 heads x 64, s] against
    Wout row-slices; evicted via ScalarE and DMA'd to DRAM.
"""

import sys

for _p in ("/opt/trn_rl_repo", "/root/.axon_site/_ro/trn_rl_repo"):
    if _p not in sys.path:
        sys.path.insert(0, _p)

import numpy as np

S = 1024
H = 1024
NH = 16
D = 64
NHL = 8          # heads per core
P = 128
N_CORES = 8
LN_EPS = 1e-5
ROPE_BASE = 10000.0
SCALE = float(D) ** -0.5

_PROGRAM = None


def _rope_tables(height, width, head_dim=D):
    """Mirror of reference.rope_2d_tables in numpy float32."""
    height = int(height)
    width = int(width)
    dim_x = head_dim // 2
    dim_y = head_dim - dim_x
    inv_fx = 1.0 / (ROPE_BASE ** (np.arange(0, dim_x, 2, dtype=np.float32) / np.float32(dim_x)))
    inv_fy = 1.0 / (ROPE_BASE ** (np.arange(0, dim_y, 2, dtype=np.float32) / np.float32(dim_y)))
    fx = np.arange(width, dtype=np.float32)[:, None] * inv_fx[None, :]
    fy = np.arange(height, dtype=np.float32)[:, None] * inv_fy[None, :]
    fx = np.concatenate([fx, fx], axis=-1)  # [W, dim_x]
    fy = np.concatenate([fy, fy], axis=-1)  # [H, dim_y]
    cos = np.concatenate([
        np.broadcast_to(np.cos(fx)[None, :, :], (height, width, dim_x)),
        np.broadcast_to(np.cos(fy)[:, None, :], (height, width, dim_y)),
    ], axis=-1).reshape(height * width, head_dim).astype(np.float32)
    sin = np.concatenate([
        np.broadcast_to(np.sin(fx)[None, :, :], (height, width, dim_x)),
        np.broadcast_to(np.sin(fy)[:, None, :], (height, width, dim_y)),
    ], axis=-1).reshape(height * width, head_dim).astype(np.float32)
    return cos, sin


def _build_program(do_compile=True):
    import concourse.mybir as mybir
    import concourse.tile as tile
    from concourse import bacc
    from concourse.masks import make_identity

    f32 = mybir.dt.float32
    f32r = mybir.dt.float32r
    bf16 = mybir.dt.bfloat16
    AF = mybir.ActivationFunctionType

    nc = bacc.Bacc("TRN2")

    # ---- DRAM parameters (per-core tensors; same program on all cores) ----
    x_d = [nc.dram_tensor(f"x_s{s}", [S, H], f32, kind="ExternalInput") for s in range(2)]
    wqk_d = [nc.dram_tensor(f"wqk_s{s}", [H, 2 * NHL * D], f32r, kind="ExternalInput") for s in range(2)]
    wv_d = [nc.dram_tensor(f"wv_s{s}", [H, NHL * D], f32r, kind="ExternalInput") for s in range(2)]
    cqk_d = [nc.dram_tensor(f"cqk_s{s}", [2, 2 * NHL * D], f32r, kind="ExternalInput") for s in range(2)]
    cv_d = [nc.dram_tensor(f"cv_s{s}", [2, NHL * D], f32r, kind="ExternalInput") for s in range(2)]
    wout_d = [nc.dram_tensor(f"wout_s{s}", [NHL * D, H], f32r, kind="ExternalInput") for s in range(2)]
    cos2_d = nc.dram_tensor("cos2", [P, S], bf16, kind="ExternalInput")
    sin2_d = nc.dram_tensor("sin2", [P, S], bf16, kind="ExternalInput")  # pre-negated/shuffle-ready
    selr_d = nc.dram_tensor("selr", [16, 8 * P], f32, kind="ExternalInput")
    out_d = [nc.dram_tensor(f"out_s{s}", [S, H], f32, kind="ExternalOutput") for s in range(2)]

    with tile.TileContext(nc) as tc:
        with (
            tc.tile_pool(name="consts", bufs=1) as consts,
            tc.tile_pool(name="persist", bufs=1) as persist,
            tc.tile_pool(name="small", bufs=2) as small,
        ):
            ident = consts.tile([P, P], f32, tag="ident")
            make_identity(nc, ident)
            cos2 = consts.tile([P, S], bf16, tag="cos2")
            nc.sync.dma_start(out=cos2, in_=cos2_d[:])
            sin2 = consts.tile([P, S], bf16, tag="sin2")
            nc.sync.dma_start(out=sin2, in_=sin2_d[:])
            selr = consts.tile([16, 8, P], f32, tag="selr")
            nc.sync.dma_start(out=selr, in_=selr_d[:].rearrange("a (t p) -> a t p", p=P))
            epsc = consts.tile([P, 1], f32, tag="epsc")
            nc.vector.memset(epsc, LN_EPS)
            zeroc = consts.tile([P, 1], f32, tag="zeroc")
            nc.vector.memset(zeroc, 0.0)

            # persistent per-stream state (live across prep -> attention -> tail)
            qkT = [[persist.tile([P, S], bf16, tag=f"qkT{s}_{nt}", name=f"qkT{s}_{nt}")
                    for nt in range(8)] for s in range(2)]
            v_sb = [[persist.tile([P, NHL, D + 1], bf16, tag=f"v{s}_{st}", name=f"v{s}_{st}")
                     for st in range(8)] for s in range(2)]

            # ---------------- prep: per stream ----------------
            with (
                tc.tile_pool(name="prep", bufs=1) as prep,
                tc.tile_pool(name="prep_psum", bufs=8, space="PSUM") as pp,
            ):
                for s in range(2):
                    stack = prep.tile([P, 17], f32, tag="stack", bufs=2, name="stack")
                    nc.vector.memset(stack[:, 16:17], 1.0)
                    xh = [prep.tile([P, S], f32r, tag=f"xh{s}_{hc}", name=f"xh{s}_{hc}") for hc in range(8)]
                    xts = []
                    for st in range(8):
                        xt = prep.tile([P, H], f32, tag=f"xt{st}", bufs=1, name=f"xt{st}")
                        xts.append(xt)
                        nc.sync.dma_start(out=xt, in_=x_d[s][st * P:(st + 1) * P, :])
                        bs = small.tile([P, 2, 6], f32, tag="bs", name="bs")
                        nc.vector.bn_stats(out=bs[:, 0, :], in_=xt[:, 0:512])
                        nc.vector.bn_stats(out=bs[:, 1, :], in_=xt[:, 512:1024])
                        mv = small.tile([P, 2], f32, tag="mv", name="mv")
                        nc.vector.bn_aggr(out=mv, in_=bs)
                        sd = small.tile([P, 1], f32, tag="sd", name="sd")
                        nc.scalar.activation(out=sd, in_=mv[:, 1:2], func=AF.Sqrt, bias=epsc)
                        nc.vector.reciprocal(out=stack[:, st:st + 1], in_=sd)
                        nc.vector.tensor_mul(stack[:, 8 + st:9 + st], stack[:, st:st + 1], mv[:, 0:1])

                    # transpose [r|rmu] columns -> rows
                    ps16 = pp.tile([17, P], f32, tag="mm", name="ps16")
                    nc.tensor.transpose(ps16, stack, ident)
                    sb16 = small.tile([17, P], f32, tag="sb16", name="sb16")
                    nc.vector.tensor_copy(out=sb16, in_=ps16)

                    rmu1 = prep.tile([2, S], f32r, tag="rmu1", bufs=2, name="rmu1")
                    for st in range(8):
                        nc.gpsimd.dma_start(out=rmu1[0:1, st * P:(st + 1) * P], in_=sb16[8 + st:9 + st, :].bitcast(f32r))
                        nc.gpsimd.dma_start(out=rmu1[1:2, st * P:(st + 1) * P], in_=sb16[16:17, :].bitcast(f32r))

                    # rbar [128, S]: r broadcast across partitions (PE selector matmuls)
                    rbar = prep.tile([P, S], f32, tag="rbar", bufs=2, name="rbar")
                    for half in range(2):
                        psb = pp.tile([P, 512], f32, tag="mm", name="psb")
                        for j in range(4):
                            st = half * 4 + j
                            nc.tensor.matmul(psb[:, j * P:(j + 1) * P], selr[:, st, :], sb16[0:16, :])
                        nc.vector.tensor_copy(out=rbar[:, half * 512:(half + 1) * 512], in_=psb)

                    for st in range(8):
                        for hc in range(8):
                            pst = pp.tile([P, P], f32, tag="mm", name="pst")
                            nc.tensor.transpose(pst, xts[st][:, hc * P:(hc + 1) * P], ident)
                            nc.vector.tensor_mul(xh[hc][:, st * P:(st + 1) * P], pst, rbar[:, st * P:(st + 1) * P])

                    # corr rows in SBUF
                    cqk_sb = prep.tile([2, 2 * NHL * D], f32r, tag="cqk", bufs=2, name="cqk_sb")
                    nc.sync.dma_start(out=cqk_sb, in_=cqk_d[s][:])
                    cv_sb = prep.tile([2, NHL * D], f32r, tag="cv", bufs=2, name="cv_sb")
                    nc.sync.dma_start(out=cv_sb, in_=cv_d[s][:])

                    # qkT: projection with transposed output [n, s]
                    for nts in ([0, 1, 2, 3], [4, 5, 6, 7]):
                        psq = [[pp.tile([P, 512], f32, tag="mm", name="psq") for sc in range(2)]
                               for j in range(len(nts))]
                        for kc in range(8):
                            wq = prep.tile([P, 2 * NHL * D], f32r, tag="wqk", bufs=2, name="wq")
                            nc.sync.dma_start(out=wq, in_=wqk_d[s][kc * P:(kc + 1) * P, :])
                            for j, nt in enumerate(nts):
                                for sc in range(2):
                                    nc.tensor.matmul(
                                        psq[j][sc],
                                        wq[:, nt * P:(nt + 1) * P],
                                        xh[kc][:, sc * 512:(sc + 1) * 512],
                                        start=(kc == 0), stop=False,
                                    )
                        for j, nt in enumerate(nts):
                            for sc in range(2):
                                nc.tensor.matmul(
                                    psq[j][sc],
                                    cqk_sb[:, nt * P:(nt + 1) * P],
                                    rmu1[:, sc * 512:(sc + 1) * 512],
                                    start=False, stop=True,
                                )
                                nc.scalar.copy(out=qkT[s][nt][:, sc * 512:(sc + 1) * 512], in_=psq[j][sc])

                    # V natural [s, n] + ones column, bf16
                    for sts in ([0, 1, 2, 3, 4, 5, 6, 7],):
                        psv = [pp.tile([P, 512], f32, tag="mm", name="psv") for st in sts]
                        for kc in range(8):
                            wvt = prep.tile([P, NHL * D], f32r, tag="wv", bufs=2, name="wvt")
                            nc.scalar.dma_start(out=wvt, in_=wv_d[s][kc * P:(kc + 1) * P, :])
                            for j, st in enumerate(sts):
                                nc.tensor.matmul(
                                    psv[j],
                                    xh[kc][:, st * P:(st + 1) * P],
                                    wvt,
                                    start=(kc == 0), stop=False,
                                )
                        for j, st in enumerate(sts):
                            nc.tensor.matmul(
                                psv[j],
                                rmu1[:, st * P:(st + 1) * P],
                                cv_sb,
                                start=False, stop=True,
                            )
                            nc.vector.memset(v_sb[s][st][:, :, D:D + 1], 1.0)
                            nc.scalar.copy(
                                out=v_sb[s][st][:, :, 0:D],
                                in_=psv[j].rearrange("p (h d) -> p h d", d=D),
                            )

                    # RoPE on all 8 qkT tiles (q tiles 0-3, k tiles 4-7)
                    for nt in range(8):
                        rot = prep.tile([P, S], bf16, tag="rot", bufs=2, name="rot")
                        for blk in (0, 64):
                            nc.gpsimd.dma_start(out=rot[blk:blk + 32, :], in_=qkT[s][nt][blk + 32:blk + 64, :])
                            nc.gpsimd.dma_start(out=rot[blk + 32:blk + 64, :], in_=qkT[s][nt][blk:blk + 32, :])
                        tmp = prep.tile([P, S], bf16, tag="ropetmp", bufs=2, name="tmp")
                        nc.gpsimd.tensor_mul(tmp, rot, sin2)
                        qc = prep.tile([P, S], bf16, tag="ropeqc", bufs=2, name="qc")
                        nc.vector.tensor_mul(qc, qkT[s][nt], cos2)
                        nc.vector.tensor_add(qkT[s][nt], qc, tmp)

            # ---------------- attention + tail ----------------
            with tc.tile_pool(name="att", bufs=1) as att:
              attn = [[att.tile([P, S], f32r, tag=f"attn{s}_{p}", name=f"attn{s}_{p}")
                       for p in range(4)] for s in range(2)]
              wo_t = [[att.tile([P, H], f32r, tag=f"wo{s}_{p}", name=f"wo{s}_{p}")
                       for p in range(4)] for s in range(2)]
              for s in range(2):
                  for p in range(4):
                      nc.sync.dma_start(out=wo_t[s][p], in_=wout_d[s][p * P:(p + 1) * P, :])
              with (
                tc.tile_pool(name="att_psum", bufs=2, space="PSUM") as ap,
                tc.tile_pool(name="acc_psum", bufs=4, space="PSUM") as accp,
              ):
                for s in range(2):
                    for p in range(4):
                        A = [[accp.tile([D + 1, 512], f32, tag="acc", name="acc")
                              for sc in range(2)] for par in range(2)]
                        for tc_i in range(16):
                            ts, tst = tc_i // 8, tc_i % 8
                            for par in range(2):
                                h = 2 * p + par
                                Pp = ap.tile([P, S], f32, tag="P", name="Pp")
                                for sc in range(2):
                                    nc.tensor.matmul(
                                        Pp[:, sc * 512:(sc + 1) * 512],
                                        qkT[ts][4 + p][64 * par:64 * par + 64, tst * P:(tst + 1) * P],
                                        qkT[s][p][64 * par:64 * par + 64, sc * 512:(sc + 1) * 512],
                                    )
                                es = att.tile([P, S], bf16, tag="exp", bufs=6, name="es")
                                nc.scalar.activation(out=es, in_=Pp, func=AF.Exp, bias=zeroc, scale=SCALE)
                                for sc in range(2):
                                    nc.tensor.matmul(
                                        A[par][sc],
                                        v_sb[ts][tst][:, h, :],
                                        es[:, sc * 512:(sc + 1) * 512],
                                        start=(tc_i == 0), stop=(tc_i == 15),
                                    )
                        for par in range(2):
                            for sc in range(2):
                                rstg = att.tile([1, 512], f32, tag="rstg", bufs=4, name="rstg")
                                nc.vector.reciprocal(out=rstg, in_=A[par][sc][D:D + 1, :])
                                rbc = att.tile([D, 512], f32, tag="rbc", bufs=4, name="rbc")
                                nc.gpsimd.partition_broadcast(rbc, rstg)
                                nc.vector.tensor_mul(
                                    attn[s][p][64 * par:64 * par + 64, sc * 512:(sc + 1) * 512],
                                    A[par][sc][0:D, :],
                                    rbc,
                                )

              # ---------------- normalize + out-projection ----------------
              with tc.tile_pool(name="tail_psum", bufs=1, space="PSUM") as tp:
                  for s in range(2):
                      for st in range(8):
                          for oc in range(2):
                              pso = tp.tile([P, 512], f32, tag="po", bufs=6, name="pso")
                              for p in range(4):
                                  nc.tensor.matmul(
                                      pso,
                                      attn[s][p][:, st * P:(st + 1) * P],
                                      wo_t[s][p][:, oc * 512:(oc + 1) * 512],
                                      start=(p == 0), stop=(p == 3),
                                  )
                              osb = att.tile([P, 512], f32, tag="osb", bufs=6, name="osb")
                              if (st + oc) % 2 == 0:
                                  nc.scalar.copy(out=osb, in_=pso)
                              else:
                                  nc.vector.tensor_copy(out=osb, in_=pso)
                              (nc.gpsimd if (st + oc) % 2 == 0 else nc.sync).dma_start(
                                  out=out_d[s][st * P:(st + 1) * P, oc * 512:(oc + 1) * 512], in_=osb)

    if do_compile:
        nc.compile()
    return nc


def _get_program():
    global _PROGRAM
    if _PROGRAM is None:
        _PROGRAM = _build_program()
    return _PROGRAM


def _host_prep(x_a, x_b, Wqkv_a, Wqkv_b, Wout_a, Wout_b,
               gamma_a, beta_a, gamma_b, beta_b, height, width):
    """Build the 8 per-core input maps."""
    cos, sin = _rope_tables(height, width)      # [S, 64]
    cosT = np.ascontiguousarray(cos.T)          # [64, S]
    sinT = sin.T.copy()
    # rot_unsigned rows: [q(32:64); q(0:32)]; signs folded into the table:
    # rows 0:32 multiply -sin[0:32], rows 32:64 multiply +sin[32:64]
    sin_sh = sinT.copy()
    sin_sh[0:32, :] *= -1.0
    import ml_dtypes
    cos2 = np.ascontiguousarray(np.concatenate([cosT, cosT], axis=0).astype(ml_dtypes.bfloat16))
    sin2 = np.ascontiguousarray(np.concatenate([sin_sh, sin_sh], axis=0).astype(ml_dtypes.bfloat16))

    selr = np.zeros((16, 8, P), np.float32)
    for t in range(8):
        selr[t, t, :] = 1.0
    selr = np.ascontiguousarray(selr.reshape(16, 8 * P))

    streams = []
    for (W, Wo, g, b) in ((Wqkv_a, Wout_a, gamma_a, beta_a), (Wqkv_b, Wout_b, gamma_b, beta_b)):
        Wg = (W * g[:, None]).astype(np.float32)       # gamma-folded
        cfull = (b.astype(np.float64) @ W.astype(np.float64)).astype(np.float32)  # beta@W [3H]
        W4 = Wg.reshape(H, 3, NH, D)
        c4 = cfull.reshape(3, NH, D)
        per_hg = []
        for hg in range(2):
            hs = slice(hg * NHL, (hg + 1) * NHL)
            wqk = np.ascontiguousarray(
                np.concatenate([W4[:, 0, hs, :].reshape(H, NHL * D),
                                W4[:, 1, hs, :].reshape(H, NHL * D)], axis=1))
            wv = np.ascontiguousarray(W4[:, 2, hs, :].reshape(H, NHL * D))
            cqk = np.concatenate([c4[0, hs, :].reshape(NHL * D), c4[1, hs, :].reshape(NHL * D)])
            cv = c4[2, hs, :].reshape(NHL * D)
            corr_qk = np.ascontiguousarray(np.stack([-wqk.sum(axis=0), cqk]).astype(np.float32))
            corr_v = np.ascontiguousarray(np.stack([-wv.sum(axis=0), cv]).astype(np.float32))
            wout = np.ascontiguousarray(Wo.reshape(NH, D, H)[hs].reshape(NHL * D, H).astype(np.float32))
            per_hg.append(dict(wqk=wqk, wv=wv, cqk=corr_qk, cv=corr_v, wout=wout))
        streams.append(per_hg)

    in_maps = []
    B = x_a.shape[0]
    for c in range(N_CORES):
        b_i, hg = (c // 2) % B, c % 2
        m = {
            "x_s0": np.ascontiguousarray(x_a[b_i]),
            "x_s1": np.ascontiguousarray(x_b[b_i]),
            "cos2": cos2, "sin2": sin2, "selr": selr,
        }
        for s in range(2):
            blk = streams[s][hg]
            m[f"wqk_s{s}"] = blk["wqk"]
            m[f"wv_s{s}"] = blk["wv"]
            m[f"cqk_s{s}"] = blk["cqk"]
            m[f"cv_s{s}"] = blk["cv"]
            m[f"wout_s{s}"] = blk["wout"]
        in_maps.append(m)
    return in_maps


def kernel(x_a, x_b, Wqkv_a, Wqkv_b, Wout_a, Wout_b,
           gamma_a, beta_a, gamma_b, beta_b, height, width):
    from concourse.bass_utils import run_bass_kernel_spmd

    x_a = np.asarray(x_a, dtype=np.float32)
    x_b = np.asarray(x_b, dtype=np.float32)
    B = x_a.shape[0]
    in_maps = _host_prep(x_a, x_b,
                         np.asarray(Wqkv_a, np.float32), np.asarray(Wqkv_b, np.float32),
                         np.asarray(Wout_a, np.float32), np.asarray(Wout_b, np.float32),
                         np.asarray(gamma_a, np.float32), np.asarray(beta_a, np.float32),
                         np.asarray(gamma_b, np.float32), np.asarray(beta_b, np.float32),
                         height, width)
    nc = _get_program()
    res = run_bass_kernel_spmd(nc, in_maps, list(range(N_CORES))).results
    out_a = np.empty((B, S, H), np.float32)
    out_b = np.empty((B, S, H), np.float32)
    for b_i in range(B):
        out_a[b_i] = res[2 * b_i]["out_s0"] + res[2 * b_i + 1]["out_s0"]
        out_b[b_i] = res[2 * b_i]["out_s1"] + res[2 * b_i + 1]["out_s1"]
    return out_a, out_b

